# revision 9
# baseline (speedup 1.0000x reference)
"""Trainium2 Bass kernel for CollisionDistanceEvaluator (segment_reduce).

Contract: kernel(**inputs) takes FULL inputs (trans [4096,3] f32,
quat [4096,4] f32, pc [4096,4096,3] f32) and returns the FULL output
[4096,1] f32, running the heavy per-point work on 8 NeuronCores
(pure data-parallel over the batch dim, 512 batches/core).

Math: reference rotates pc by inv(quat) (unit norm -> rotation R),
translates by -trans, tests an axis-aligned box (center C, half ext H),
and takes the per-batch masked mean of point norms.  Host precomputes
    A[b] = R[b] / H[:,None]       (box-normalized rotation rows)
    o[b] = -(trans[b] + C) / H
Device, per point p (v_i = A_i.p + o_i, the box coords):
    mask = max(v_x^2, v_y^2, |v_z|) <= 1
    n2/Hx^2 = v_x^2 + ky*v_y^2 + kz*(v_z^2 + (2Cz/Hz)*v_z) + cc
    out[b]  = -10000*Hx*sum(mask*sqrt(n2~))/max(cnt,1)  (+10000 if cnt==0)

Engine split per 128-batch tile, per 1024-pt chunk (4 tiles x 4 chunks):
  PE     : 18 diag matmuls (z,x,y plane order) -> PSUM
  ACT    : vz = Identity(uz+oz); qx,qy = Square(u+o); Sqrt(mn)+accum
           (sqrt issued one chunk late so ACT never blocks on DVE)
  DVE    : max(qx,qy); abs_max(.,vz); 3x stt n2-chain; mn stt
  GPSIMD : qz = vz*vz (tensor_tensor); mask = is_le(mx,1) + count accum
"""

import numpy as np

import concourse.bass as bass
import concourse.bacc as bacc
import concourse.mybir as mybir
from concourse.tile import TileContext
from concourse.bass_utils import run_bass_kernel_spmd
from concourse import library_config


def _ensure_ntff_hook():
    """Register the axon NTFF profile hook if the image's antenv lacks it."""
    import sys
    import types
    try:
        from antenv.axon_hooks import get_axon_ntff_profile_hook  # noqa
        return
    except ImportError:
        pass
    try:
        import antenv
        from trn_agent_boot.trn_boot import _ntff_profile_via_ctypes
        mod = types.ModuleType("antenv.axon_hooks")
        mod._hook = _ntff_profile_via_ctypes("/opt/axon/libaxon_pjrt.so")

        def set_axon_ntff_profile_hook(h):
            mod._hook = h

        def get_axon_ntff_profile_hook():
            return mod._hook

        mod.set_axon_ntff_profile_hook = set_axon_ntff_profile_hook
        mod.get_axon_ntff_profile_hook = get_axon_ntff_profile_hook
        sys.modules["antenv.axon_hooks"] = mod
        antenv.axon_hooks = mod
    except Exception:
        pass


_ensure_ntff_hook()

N_CORES = 8
B_FULL, N_PTS = 4096, 4096
B_CORE = B_FULL // N_CORES          # 512
N_TILES = B_CORE // 128             # 4
CHUNK = 1024                         # points per pipeline chunk
N_CHUNKS = N_PTS // CHUNK           # 4
MM = 512                             # matmul moving max

DIST_THRESHOLD = 0.001
DIST_COEFF = 10000.0
BOX_CENTER = np.array([-0.001782, 1.005e-05, 0.0431621], dtype=np.float64)
HALF_EXT = np.array([
    0.204416 / 2 + DIST_THRESHOLD,
    0.0632517 / 2 + DIST_THRESHOLD,
    0.1381738 / 2 + DIST_THRESHOLD,
], dtype=np.float64)

# n2 normalization constants (n2 / Hx^2)
_HX2 = HALF_EXT[0] * HALF_EXT[0]
K_Y = float((HALF_EXT[1] / HALF_EXT[0]) ** 2)
K_Z = float((HALF_EXT[2] / HALF_EXT[0]) ** 2)
KLKZ = float(2.0 * BOX_CENTER[2] / HALF_EXT[2])     # v_z linear / kz
EPS_PAD = 1.5e-3
CC = float((BOX_CENTER[0] ** 2 + BOX_CENTER[1] ** 2 + BOX_CENTER[2] ** 2)
           / _HX2 + EPS_PAD)
OUT_SCALE = float(-DIST_COEFF * HALF_EXT[0])

_NC_CACHE = {}


def _build_bass():
    f16, f32 = mybir.dt.float16, mybir.dt.float32
    Alu = mybir.AluOpType
    Act = mybir.ActivationFunctionType

    nc = bacc.Bacc()
    xyz = nc.declare_dram_parameter(
        "xyz", [3, B_CORE, N_PTS], f16, isOutput=False)
    # per-batch scalars: cols 0-2 = o_i
    coef = nc.declare_dram_parameter(
        "coef", [N_TILES, 128, 8], f32, isOutput=False)
    # diagonal PE weights: [tile, i, c] -> diag(A[batch, i, c])
    wd = nc.declare_dram_parameter(
        "wd", [N_TILES, 128, 9 * 128], f16, isOutput=False)
    out = nc.declare_dram_parameter("out", [B_CORE, 1], f32, isOutput=True)
    xyz_ap, coef_ap, wd_ap, out_ap = xyz[:], coef[:], wd[:], out[:]

    with TileContext(nc) as tc, \
            tc.tile_pool(name="data", bufs=2) as data, \
            tc.tile_pool(name="vzp", bufs=2) as vzp, \
            tc.tile_pool(name="ck", bufs=3) as ck, \
            tc.tile_pool(name="wpool", bufs=2) as wpool, \
            tc.tile_pool(name="psum", bufs=1, space="PSUM") as psum, \
            tc.tile_pool(name="small", bufs=4) as small:
        # PE warm-up: junk matmuls during the initial DMA ramp keep the
        # HAM clock gate open so the first real matmuls run at 2.4 GHz
        wu = wpool.tile([128, 128], f16, tag="warm")
        nc.vector.memset(wu[:], 0.0)
        pwu = psum.tile([128, MM], f32, tag="warm")
        for k in range(96):
            nc.tensor.matmul(pwu[:, :128], wu[:], wu[:, :],
                             start=True, stop=True)

        for t in range(N_TILES):
            r = slice(t * 128, (t + 1) * 128)
            ct = small.tile([128, 8], f32, tag="coef")
            nc.sync.dma_start(out=ct[:], in_=coef_ap[t, :, :])
            xt = data.tile([128, N_PTS], f16, tag="x")
            yt = data.tile([128, N_PTS], f16, tag="y")
            zt = data.tile([128, N_PTS], f16, tag="z")
            half = N_PTS // 2
            h0 = slice(0, half)
            nc.sync.dma_start(out=zt[:, h0], in_=xyz_ap[2, r, h0])
            nc.sync.dma_start(out=xt[:, h0], in_=xyz_ap[0, r, h0])
            nc.sync.dma_start(out=yt[:, h0], in_=xyz_ap[1, r, h0])
            wt = wpool.tile([128, 9 * 128], f16, tag="wt")
            nc.sync.dma_start(out=wt[:], in_=wd_ap[t, :, :])
            h1 = slice(half, N_PTS)
            nc.sync.dma_start(out=zt[:, h1], in_=xyz_ap[2, r, h1])
            nc.sync.dma_start(out=xt[:, h1], in_=xyz_ap[0, r, h1])
            nc.sync.dma_start(out=yt[:, h1], in_=xyz_ap[1, r, h1])
            planes = (xt, yt, zt)

            def wsel(i, c, wt=wt):
                w = 3 * i + c
                return wt[:, w * 128:(w + 1) * 128]

            # full-width vz (drained box-z coords, reused by 3 consumers)
            vzt = vzp.tile([128, N_PTS], f16, tag="vz")
            rcv = small.tile([128, N_CHUNKS], f32, tag="rcv")
            rsv = small.tile([128, N_CHUNKS], f32, tag="rsv")

            prev = None  # delayed-sqrt pipeline register
            for j in range(N_CHUNKS):
                cs = slice(j * CHUNK, (j + 1) * CHUNK)
                uz = psum.tile([128, CHUNK], f32, tag="uz")
                ux = psum.tile([128, CHUNK], f32, tag="ux")
                uy = psum.tile([128, CHUNK], f32, tag="uy")
                # PE: z first (its drain feeds GPSIMD + the DVE chain)
                for ut, i in ((uz, 2), (ux, 0), (uy, 1)):
                    for h in range(CHUNK // MM):
                        hs = slice(h * MM, (h + 1) * MM)
                        ps = slice(j * CHUNK + h * MM,
                                   j * CHUNK + (h + 1) * MM)
                        for c in range(3):
                            nc.tensor.matmul(
                                ut[:, hs], wsel(i, c), planes[c][:, ps],
                                start=(c == 0), stop=(c == 2))
                # ACT drains (PSUM -> SBUF f16)
                nc.scalar.activation(
                    vzt[:, cs], uz[:], Act.Identity, bias=ct[:, 2:3])
                qx = ck.tile([128, CHUNK], f16, tag="qx")
                nc.scalar.activation(
                    qx[:], ux[:], Act.Square, bias=ct[:, 0:1])
                qy = ck.tile([128, CHUNK], f16, tag="qy")
                nc.scalar.activation(
                    qy[:], uy[:], Act.Square, bias=ct[:, 1:2])
                # DVE: qz = vz*vz ; mask chain with count accum
                qz = ck.tile([128, CHUNK], f16, tag="qz")
                nc.vector.tensor_tensor(qz[:], vzt[:, cs], vzt[:, cs],
                                        Alu.mult)
                mx = ck.tile([128, CHUNK], f16, tag="mx")
                nc.vector.tensor_tensor(mx[:], qx[:], qy[:], Alu.max)
                nc.vector.tensor_tensor(mx[:], mx[:], qz[:], Alu.max)
                mask = ck.tile([128, CHUNK], f16, tag="mask")
                nc.vector.tensor_scalar(
                    mask[:], mx[:], 1.0, 0.0, Alu.is_le, Alu.add,
                    accum_out=rcv[:, j:j + 1])
                # DVE n2 chain (normalized by Hx^2):
                # e = kz*(qz + klkz*vz) + qx ; e += ky*qy ; mn = (e+cc)*mask
                e = ck.tile([128, CHUNK], f16, tag="e")
                nc.vector.scalar_tensor_tensor(
                    e[:], vzt[:, cs], KLKZ, qz[:], Alu.mult, Alu.add)
                nc.vector.scalar_tensor_tensor(
                    e[:], e[:], K_Z, qx[:], Alu.mult, Alu.add)
                nc.vector.scalar_tensor_tensor(
                    e[:], qy[:], K_Y, e[:], Alu.mult, Alu.add)
                mn = ck.tile([128, CHUNK], f16, tag="mn")
                nc.vector.scalar_tensor_tensor(
                    mn[:], e[:], CC, mask[:], Alu.add, Alu.mult)
                # ACT sqrt of the PREVIOUS chunk (keeps ACT ahead of DVE)
                if prev is not None:
                    pmn, pj = prev
                    sq = ck.tile([128, CHUNK], f16, tag="sq")
                    nc.scalar.activation(
                        sq[:], pmn[:], Act.Sqrt,
                        accum_out=rsv[:, pj:pj + 1])
                prev = (mn, j)
            pmn, pj = prev
            sq = ck.tile([128, CHUNK], f16, tag="sq")
            nc.scalar.activation(
                sq[:], pmn[:], Act.Sqrt, accum_out=rsv[:, pj:pj + 1])

            # tail: out = (cnt==0)*10000 + OUT_SCALE*rs/max(cnt,1)
            rc = small.tile([128, 1], f32, tag="rc")
            rs = small.tile([128, 1], f32, tag="rs")
            nc.vector.tensor_reduce(
                rc[:], rcv[:], mybir.AxisListType.X, Alu.add)
            nc.vector.tensor_reduce(
                rs[:], rsv[:], mybir.AxisListType.X, Alu.add)
            rc1 = small.tile([128, 1], f32, tag="rc1")
            nc.vector.tensor_scalar(rc1[:], rc[:], 1.0, None, Alu.max)
            inv = small.tile([128, 1], f32, tag="inv")
            nc.vector.reciprocal(inv[:], rc1[:])
            val = small.tile([128, 1], f32, tag="val")
            nc.vector.scalar_tensor_tensor(
                val[:], rs[:], OUT_SCALE, inv[:], Alu.mult, Alu.mult)
            zer = small.tile([128, 1], f32, tag="zer")
            nc.vector.tensor_scalar(zer[:], rc[:], 0.0, None, Alu.is_le)
            ot = small.tile([128, 1], f32, tag="ot")
            nc.vector.scalar_tensor_tensor(
                ot[:], zer[:], DIST_COEFF, val[:], Alu.mult, Alu.add)
            nc.sync.dma_start(out=out_ap[r, :], in_=ot[:])
    nc.compile()
    return nc


def _get_nc():
    if "nc" not in _NC_CACHE:
        _NC_CACHE["nc"] = _build_bass()
    return _NC_CACHE["nc"]


def _host_coefficients(trans, quat):
    """Per-batch A = R/H [B,3,3] and o = -(t+C)/H [B,3] (computed in f64)."""
    q = np.asarray(quat, np.float64)
    t = np.asarray(trans, np.float64)
    B = q.shape[0]
    s = (q * q).sum(-1)
    qi = np.concatenate([-q[:, :3], q[:, 3:]], -1) / s[:, None]
    v, w = qi[:, :3], qi[:, 3]
    vv = v[:, :, None] * v[:, None, :]
    w2mv = w * w - (v * v).sum(-1)
    Vx = np.zeros((B, 3, 3))
    Vx[:, 0, 1] = -v[:, 2]
    Vx[:, 0, 2] = v[:, 1]
    Vx[:, 1, 0] = v[:, 2]
    Vx[:, 1, 2] = -v[:, 0]
    Vx[:, 2, 0] = -v[:, 1]
    Vx[:, 2, 1] = v[:, 0]
    R = (w2mv[:, None, None] * np.eye(3)
         + 2.0 * vv
         + 2.0 * w[:, None, None] * Vx)
    A = R / HALF_EXT[None, :, None]
    o = -(t + BOX_CENTER[None, :]) / HALF_EXT[None, :]
    return A.astype(np.float32), o.astype(np.float32)


def _make_in_maps(trans, quat, pc):
    A, o = _host_coefficients(trans, quat)
    coef_full = np.concatenate(
        [o, np.zeros((B_FULL, 5), np.float32)], axis=1)  # [B,8]
    # planar fp16 [3, B, N]
    pcT = np.ascontiguousarray(
        np.asarray(pc, np.float32).transpose(2, 0, 1)).astype(np.float16)
    # diagonal weights [tile, i, c] per core
    idx = np.arange(128)
    in_maps = []
    for cidx in range(N_CORES):
        bs, be = cidx * B_CORE, (cidx + 1) * B_CORE
        Ac = A[bs:be].reshape(N_TILES, 128, 3, 3)
        wdc = np.zeros((N_TILES, 3, 3, 128, 128), np.float16)
        wdc[:, :, :, idx, idx] = np.transpose(
            Ac, (0, 2, 3, 1)).astype(np.float16)
        in_maps.append({
            "xyz": np.ascontiguousarray(pcT[:, bs:be, :]),
            "coef": np.ascontiguousarray(
                coef_full[bs:be].reshape(N_TILES, 128, 8)),
            "wd": np.ascontiguousarray(
                np.transpose(wdc.reshape(N_TILES, 9, 128, 128),
                             (0, 2, 1, 3)).reshape(N_TILES, 128, 9 * 128)),
        })
    return in_maps


def run_spmd(trans, quat, pc, **spmd_kwargs):
    """Shard, run on 8 cores, gather. Returns (output, BassKernelResults)."""
    in_maps = _make_in_maps(trans, quat, pc)
    res = run_bass_kernel_spmd(
        _get_nc(), in_maps, list(range(N_CORES)), **spmd_kwargs)
    outs = [res.results[i]["out"] for i in range(N_CORES)]
    full = np.concatenate(outs, axis=0).astype(np.float32)
    return full, res


def kernel(trans, quat, pc):
    full, _ = run_spmd(trans, quat, pc)
    return full


# revision 16
# speedup vs baseline: 1.1128x; 1.1128x over previous
"""Trainium2 Bass kernel for CollisionDistanceEvaluator (segment_reduce).

Contract: kernel(**inputs) takes FULL inputs (trans [4096,3] f32,
quat [4096,4] f32, pc [4096,4096,3] f32) and returns the FULL output
[4096,1] f32, running the heavy per-point work on 8 NeuronCores
(pure data-parallel over the batch dim, 512 batches/core).

Math: reference rotates pc by inv(quat) (unit norm -> rotation R),
translates by -trans, tests an axis-aligned box (center C, half ext H),
and takes the per-batch masked mean of point norms.  Host precomputes
    A[b] = R[b] / H[:,None]       (box-normalized rotation rows)
    o[b] = -(trans[b] + C) / H
Device, per point p (v_i = A_i.p + o_i, the box coords):
    mask = max(v_x^2, v_y^2, |v_z|) <= 1
    n2/Hx^2 = v_x^2 + ky*v_y^2 + kz*(v_z^2 + (2Cz/Hz)*v_z) + cc
    out[b]  = -10000*Hx*sum(mask*sqrt(n2~))/max(cnt,1)  (+10000 if cnt==0)

Engine split per 128-batch tile, per 1024-pt chunk (4 tiles x 4 chunks):
  PE     : 18 diag matmuls (z,x,y plane order) -> PSUM
  ACT    : vz = Identity(uz+oz); qx,qy = Square(u+o); Sqrt(mn)+accum
           (sqrt issued one chunk late so ACT never blocks on DVE)
  DVE    : max(qx,qy); abs_max(.,vz); 3x stt n2-chain; mn stt
  GPSIMD : qz = vz*vz (tensor_tensor); mask = is_le(mx,1) + count accum
"""

import numpy as np

import concourse.bass as bass
import concourse.bacc as bacc
import concourse.mybir as mybir
from concourse.tile import TileContext
from concourse.bass_utils import run_bass_kernel_spmd
from concourse import library_config


def _ensure_ntff_hook():
    """Register the axon NTFF profile hook if the image's antenv lacks it."""
    import sys
    import types
    try:
        from antenv.axon_hooks import get_axon_ntff_profile_hook  # noqa
        return
    except ImportError:
        pass
    try:
        import antenv
        from trn_agent_boot.trn_boot import _ntff_profile_via_ctypes
        mod = types.ModuleType("antenv.axon_hooks")
        mod._hook = _ntff_profile_via_ctypes("/opt/axon/libaxon_pjrt.so")

        def set_axon_ntff_profile_hook(h):
            mod._hook = h

        def get_axon_ntff_profile_hook():
            return mod._hook

        mod.set_axon_ntff_profile_hook = set_axon_ntff_profile_hook
        mod.get_axon_ntff_profile_hook = get_axon_ntff_profile_hook
        sys.modules["antenv.axon_hooks"] = mod
        antenv.axon_hooks = mod
    except Exception:
        pass


_ensure_ntff_hook()

N_CORES = 8
B_FULL, N_PTS = 4096, 4096
B_CORE = B_FULL // N_CORES          # 512
N_TILES = B_CORE // 128             # 4
CHUNK = 1024                         # points per pipeline chunk
N_CHUNKS = N_PTS // CHUNK           # 4
MM = 512                             # matmul moving max

DIST_THRESHOLD = 0.001
DIST_COEFF = 10000.0
BOX_CENTER = np.array([-0.001782, 1.005e-05, 0.0431621], dtype=np.float64)
HALF_EXT = np.array([
    0.204416 / 2 + DIST_THRESHOLD,
    0.0632517 / 2 + DIST_THRESHOLD,
    0.1381738 / 2 + DIST_THRESHOLD,
], dtype=np.float64)

# n2 normalization constants (n2 / Hx^2)
_HX2 = HALF_EXT[0] * HALF_EXT[0]
K_Y = float((HALF_EXT[1] / HALF_EXT[0]) ** 2)
K_Z = float((HALF_EXT[2] / HALF_EXT[0]) ** 2)
KL = float(2.0 * HALF_EXT[2] * BOX_CENTER[2] / _HX2)  # v_z linear coeff
EPS_PAD = 1.5e-3
CC = float((BOX_CENTER[0] ** 2 + BOX_CENTER[1] ** 2 + BOX_CENTER[2] ** 2)
           / _HX2 + EPS_PAD)
CNT_BIG = 2048.0     # mask sentinel; n2~ << 2048 and 4096*2048 < 2^24
OUT_SCALE = float(-DIST_COEFF * HALF_EXT[0])

_NC_CACHE = {}


def _build_bass():
    f16, f32 = mybir.dt.float16, mybir.dt.float32
    Alu = mybir.AluOpType
    Act = mybir.ActivationFunctionType

    nc = bacc.Bacc()
    xyz = nc.declare_dram_parameter(
        "xyz", [3, B_CORE, N_PTS], f16, isOutput=False)
    # per-batch scalars: cols 0-2 = o_i
    coef = nc.declare_dram_parameter(
        "coef", [N_TILES, 128, 8], f32, isOutput=False)
    # diagonal PE weights: [tile, i, c] -> diag(A[batch, i, c])
    wd = nc.declare_dram_parameter(
        "wd", [N_TILES, 128, 9 * 128], f16, isOutput=False)
    out = nc.declare_dram_parameter("out", [B_CORE, 1], f32, isOutput=True)
    xyz_ap, coef_ap, wd_ap, out_ap = xyz[:], coef[:], wd[:], out[:]

    with TileContext(nc) as tc, \
            tc.tile_pool(name="data", bufs=2) as data, \
            tc.tile_pool(name="vzp", bufs=2) as vzp, \
            tc.tile_pool(name="ck", bufs=3) as ck, \
            tc.tile_pool(name="wpool", bufs=2) as wpool, \
            tc.tile_pool(name="psum", bufs=1, space="PSUM") as psum, \
            tc.tile_pool(name="small", bufs=4) as small:
        # PE warm-up: junk matmuls during the initial DMA ramp keep the
        # HAM clock gate open so the first real matmuls run at 2.4 GHz
        wu = wpool.tile([128, 128], f16, tag="warm")
        nc.vector.memset(wu[:], 0.0)
        pwu = psum.tile([128, MM], f32, tag="warm")
        for k in range(96):
            nc.tensor.matmul(pwu[:, :128], wu[:], wu[:, :],
                             start=True, stop=True)

        for t in range(N_TILES):
            r = slice(t * 128, (t + 1) * 128)
            ct = small.tile([128, 8], f32, tag="coef")
            nc.sync.dma_start(out=ct[:], in_=coef_ap[t, :, :])
            xt = data.tile([128, N_PTS], f16, tag="x")
            yt = data.tile([128, N_PTS], f16, tag="y")
            zt = data.tile([128, N_PTS], f16, tag="z")
            half = N_PTS // 2
            h0 = slice(0, half)
            nc.sync.dma_start(out=zt[:, h0], in_=xyz_ap[2, r, h0])
            nc.sync.dma_start(out=xt[:, h0], in_=xyz_ap[0, r, h0])
            nc.sync.dma_start(out=yt[:, h0], in_=xyz_ap[1, r, h0])
            wt = wpool.tile([128, 9 * 128], f16, tag="wt")
            nc.sync.dma_start(out=wt[:], in_=wd_ap[t, :, :])
            h1 = slice(half, N_PTS)
            nc.sync.dma_start(out=zt[:, h1], in_=xyz_ap[2, r, h1])
            nc.sync.dma_start(out=xt[:, h1], in_=xyz_ap[0, r, h1])
            nc.sync.dma_start(out=yt[:, h1], in_=xyz_ap[1, r, h1])
            planes = (xt, yt, zt)

            def wsel(i, c, wt=wt):
                w = 3 * i + c
                return wt[:, w * 128:(w + 1) * 128]

            # full-width vz (drained box-z coords, reused by 3 consumers)
            vzt = vzp.tile([128, N_PTS], f16, tag="vz")
            rcv = small.tile([128, N_CHUNKS], f32, tag="rcv")
            rsv = small.tile([128, N_CHUNKS], f32, tag="rsv")

            prev = None  # delayed-sqrt pipeline register
            for j in range(N_CHUNKS):
                cs = slice(j * CHUNK, (j + 1) * CHUNK)
                uz = psum.tile([128, CHUNK], f32, tag="uz")
                ux = psum.tile([128, CHUNK], f32, tag="ux")
                uy = psum.tile([128, CHUNK], f32, tag="uy")
                # PE: z first (its drain feeds the DVE chain); c outer so
                # each diagonal weight is loaded once per chunk (9 LDW not 18)
                for ut, i in ((uz, 2), (ux, 0), (uy, 1)):
                    for c in range(3):
                        for h in range(CHUNK // MM):
                            hs = slice(h * MM, (h + 1) * MM)
                            ps = slice(j * CHUNK + h * MM,
                                       j * CHUNK + (h + 1) * MM)
                            nc.tensor.matmul(
                                ut[:, hs], wsel(i, c), planes[c][:, ps],
                                start=(c == 0), stop=(c == 2))
                # ACT drains (PSUM -> SBUF f16)
                nc.scalar.activation(
                    vzt[:, cs], uz[:], Act.Identity, bias=ct[:, 2:3])
                qx = ck.tile([128, CHUNK], f16, tag="qx")
                nc.scalar.activation(
                    qx[:], ux[:], Act.Square, bias=ct[:, 0:1])
                qy = ck.tile([128, CHUNK], f16, tag="qy")
                nc.scalar.activation(
                    qy[:], uy[:], Act.Square, bias=ct[:, 1:2])
                # DVE: qz = vz*vz ; mask chain -> sentinel g in {0, 2048}
                # (count rides g's accum: cnt = sum(g)/2048)
                qz = ck.tile([128, CHUNK], f16, tag="qz")
                nc.vector.tensor_tensor(qz[:], vzt[:, cs], vzt[:, cs],
                                        Alu.mult)
                mx = ck.tile([128, CHUNK], f16, tag="mx")
                nc.vector.tensor_tensor(mx[:], qx[:], qy[:], Alu.max)
                nc.vector.tensor_tensor(mx[:], mx[:], qz[:], Alu.max)
                g = ck.tile([128, CHUNK], f16, tag="g")
                nc.vector.tensor_scalar(
                    g[:], mx[:], 1.0, 0.0, Alu.is_le, Alu.add,
                    accum_out=rcv[:, j:j + 1])
                # DVE n2 assembly, all 4x-mode ts + 2x TT (no 1x stt):
                # n2~ = qx + ky*qy + kz*qz + kl*vz + cc
                t1 = ck.tile([128, CHUNK], f16, tag="t1")
                nc.vector.tensor_scalar(
                    t1[:], qy[:], K_Y, None, Alu.mult)
                t2 = ck.tile([128, CHUNK], f16, tag="t2")
                nc.vector.tensor_scalar(
                    t2[:], qz[:], K_Z, None, Alu.mult)
                s3 = ck.tile([128, CHUNK], f16, tag="s3")
                nc.vector.tensor_scalar(
                    s3[:], vzt[:, cs], KL, CC, Alu.mult, Alu.add)
                nc.vector.tensor_tensor(t1[:], qx[:], t1[:], Alu.add)
                nc.vector.tensor_tensor(t2[:], t2[:], s3[:], Alu.add)
                nc.vector.tensor_tensor(t1[:], t1[:], t2[:], Alu.add)
                # masked n2 (g is the 0/1 mask)
                mn = ck.tile([128, CHUNK], f16, tag="mn")
                nc.vector.tensor_tensor(mn[:], g[:], t1[:], Alu.mult)
                # ACT sqrt of the PREVIOUS chunk (keeps ACT ahead of DVE)
                if prev is not None:
                    pmn, pj = prev
                    sq = ck.tile([128, CHUNK], f16, tag="sq")
                    nc.scalar.activation(
                        sq[:], pmn[:], Act.Sqrt,
                        accum_out=rsv[:, pj:pj + 1])
                prev = (mn, j)
            pmn, pj = prev
            sq = ck.tile([128, CHUNK], f16, tag="sq")
            nc.scalar.activation(
                sq[:], pmn[:], Act.Sqrt, accum_out=rsv[:, pj:pj + 1])

            # tail: out = (cnt==0)*10000 + OUT_SCALE*rs/max(cnt,1)
            rc = small.tile([128, 1], f32, tag="rc")
            rs = small.tile([128, 1], f32, tag="rs")
            nc.vector.tensor_reduce(
                rc[:], rcv[:], mybir.AxisListType.X, Alu.add)
            nc.vector.tensor_reduce(
                rs[:], rsv[:], mybir.AxisListType.X, Alu.add)
            rc1 = small.tile([128, 1], f32, tag="rc1")
            nc.vector.tensor_scalar(rc1[:], rc[:], 1.0, None, Alu.max)
            inv = small.tile([128, 1], f32, tag="inv")
            nc.vector.reciprocal(inv[:], rc1[:])
            val = small.tile([128, 1], f32, tag="val")
            nc.vector.scalar_tensor_tensor(
                val[:], rs[:], OUT_SCALE, inv[:], Alu.mult, Alu.mult)
            zer = small.tile([128, 1], f32, tag="zer")
            nc.vector.tensor_scalar(zer[:], rc[:], 0.0, None, Alu.is_le)
            ot = small.tile([128, 1], f32, tag="ot")
            nc.vector.scalar_tensor_tensor(
                ot[:], zer[:], DIST_COEFF, val[:], Alu.mult, Alu.add)
            nc.sync.dma_start(out=out_ap[r, :], in_=ot[:])
    nc.compile()
    return nc


def _get_nc():
    if "nc" not in _NC_CACHE:
        _NC_CACHE["nc"] = _build_bass()
    return _NC_CACHE["nc"]


def _host_coefficients(trans, quat):
    """Per-batch A = R/H [B,3,3] and o = -(t+C)/H [B,3] (computed in f64)."""
    q = np.asarray(quat, np.float64)
    t = np.asarray(trans, np.float64)
    B = q.shape[0]
    s = (q * q).sum(-1)
    qi = np.concatenate([-q[:, :3], q[:, 3:]], -1) / s[:, None]
    v, w = qi[:, :3], qi[:, 3]
    vv = v[:, :, None] * v[:, None, :]
    w2mv = w * w - (v * v).sum(-1)
    Vx = np.zeros((B, 3, 3))
    Vx[:, 0, 1] = -v[:, 2]
    Vx[:, 0, 2] = v[:, 1]
    Vx[:, 1, 0] = v[:, 2]
    Vx[:, 1, 2] = -v[:, 0]
    Vx[:, 2, 0] = -v[:, 1]
    Vx[:, 2, 1] = v[:, 0]
    R = (w2mv[:, None, None] * np.eye(3)
         + 2.0 * vv
         + 2.0 * w[:, None, None] * Vx)
    A = R / HALF_EXT[None, :, None]
    o = -(t + BOX_CENTER[None, :]) / HALF_EXT[None, :]
    return A.astype(np.float32), o.astype(np.float32)


def _make_in_maps(trans, quat, pc):
    A, o = _host_coefficients(trans, quat)
    coef_full = np.concatenate(
        [o, np.zeros((B_FULL, 5), np.float32)], axis=1)  # [B,8]
    # planar fp16 [3, B, N]
    pcT = np.ascontiguousarray(
        np.asarray(pc, np.float32).transpose(2, 0, 1)).astype(np.float16)
    # diagonal weights [tile, i, c] per core
    idx = np.arange(128)
    in_maps = []
    for cidx in range(N_CORES):
        bs, be = cidx * B_CORE, (cidx + 1) * B_CORE
        Ac = A[bs:be].reshape(N_TILES, 128, 3, 3)
        wdc = np.zeros((N_TILES, 3, 3, 128, 128), np.float16)
        wdc[:, :, :, idx, idx] = np.transpose(
            Ac, (0, 2, 3, 1)).astype(np.float16)
        in_maps.append({
            "xyz": np.ascontiguousarray(pcT[:, bs:be, :]),
            "coef": np.ascontiguousarray(
                coef_full[bs:be].reshape(N_TILES, 128, 8)),
            "wd": np.ascontiguousarray(
                np.transpose(wdc.reshape(N_TILES, 9, 128, 128),
                             (0, 2, 1, 3)).reshape(N_TILES, 128, 9 * 128)),
        })
    return in_maps


def run_spmd(trans, quat, pc, **spmd_kwargs):
    """Shard, run on 8 cores, gather. Returns (output, BassKernelResults)."""
    in_maps = _make_in_maps(trans, quat, pc)
    res = run_bass_kernel_spmd(
        _get_nc(), in_maps, list(range(N_CORES)), **spmd_kwargs)
    outs = [res.results[i]["out"] for i in range(N_CORES)]
    full = np.concatenate(outs, axis=0).astype(np.float32)
    return full, res


def kernel(trans, quat, pc):
    full, _ = run_spmd(trans, quat, pc)
    return full
